# revision 1
# baseline (speedup 1.0000x reference)
"""Self-contained Trainium2 Bass kernel for NemotronH MTP MoE layer.

Expert-parallel over 8 NeuronCores: core c owns experts [8c, 8c+8); the
shared-expert MLP is tensor-parallel sliced (256 of 2048 intermediate dims
per core).  The DeepSeekV3-style gate is computed host-side (tiny), tokens
are dispatched host-side into per-expert column blocks with the combine
weight folded in as sqrt(w) (exact: relu^2 is degree-2 homogeneous), and
each core scatter-adds its experts' outputs into its [T, H] partial with
indirect accumulate-DMA.  The host sums the 8 partials (the expert-parallel
unshard/combine).

Matmuls run in float16 (same 10-bit mantissa as the TF32/f32r path, full
PE rate, half the DMA bytes), accumulating in fp32 PSUM.
"""

import sys

sys.path.insert(0, "/opt/trn_rl_repo")

import numpy as np

# ---- problem constants (hardcoded per contract) ----
B, S, H = 2, 512, 2048
E, G, TOPK_G, K = 64, 8, 4, 6
I = 512
SH_I = 2048
RSF = 2.5
T = B * S  # 1024 tokens
N_CORES = 8
EL = E // N_CORES  # 8 experts per core
SH_SL = SH_I // N_CORES  # 256 shared-intermediate dims per core
P = 128
KH = H // P  # 16 K-tiles over hidden
KI = I // P  # 4 K-tiles over expert intermediate
OOB = 1 << 27  # padded scatter index -> skipped via bounds_check

_PROG_CACHE = {}


def _gate_numpy(x, gate_w, gate_bias):
    """noaux_tc gate: sigmoid+bias, group top-2 sum, top-4 groups, top-6."""
    logits = x @ gate_w.T
    scores = 1.0 / (1.0 + np.exp(-logits))
    scores_b = scores + gate_bias
    sb_g = scores_b.reshape(T, G, E // G)
    top2 = np.sort(sb_g, axis=-1)[..., -2:].sum(-1, dtype=np.float32)
    grp_idx = np.argsort(-top2, axis=-1, kind="stable")[:, :TOPK_G]
    grp_mask = np.zeros((T, G), np.float32)
    np.put_along_axis(grp_mask, grp_idx, 1.0, axis=1)
    expert_mask = np.repeat(grp_mask, E // G, axis=-1) > 0
    masked = np.where(expert_mask, scores_b, -np.inf)
    top_idx = np.argsort(-masked, axis=1, kind="stable")[:, :K]
    topw = np.take_along_axis(scores, top_idx, axis=1)
    topw = topw / (topw.sum(-1, keepdims=True, dtype=np.float32) + 1e-20) * RSF
    return top_idx, topw.astype(np.float32)


def _build_program(nslot):
    """Build + compile the SPMD Bass program. nslot = 128-row M-tiles per
    expert (1 unless some expert holds >128 tokens)."""
    import concourse.bass as bass
    import concourse.tile as tile
    from concourse import bacc, mybir
    from concourse.masks import make_identity

    f32 = mybir.dt.float32
    f16 = mybir.dt.float16
    Relu = mybir.ActivationFunctionType.Relu

    NV = EL * nslot  # virtual experts (one 128-token M-tile each)

    nc = bacc.Bacc("TRN2", target_bir_lowering=False, debug=False, num_devices=N_CORES)

    xt = nc.dram_tensor("xt", [H, T], f16, kind="ExternalInput").ap()
    xst = nc.dram_tensor("xst", [H, NV * P], f16, kind="ExternalInput").ap()
    w1t = nc.dram_tensor("w1t", [EL, H, I], f16, kind="ExternalInput").ap()
    w2t = nc.dram_tensor("w2t", [EL, I, H], f16, kind="ExternalInput").ap()
    shupt = nc.dram_tensor("shupt", [H, SH_SL], f16, kind="ExternalInput").ap()
    shdownt = nc.dram_tensor("shdownt", [SH_SL, H], f16, kind="ExternalInput").ap()
    out = nc.dram_tensor("out", [T, H], f32, kind="ExternalOutput").ap()
    yall = nc.dram_tensor("yall", [NV * P, H], f32, kind="ExternalOutput").ap()

    with tile.TileContext(nc) as tc:
        with (
            tc.tile_pool(name="p_xs", bufs=3) as p_xs,  # per-expert tokens
            tc.tile_pool(name="p_xt", bufs=4) as p_xt,
            tc.tile_pool(name="p_shupt", bufs=3) as p_shupt,
            tc.tile_pool(name="p_shdownt", bufs=1) as p_shdownt,  # tags sd0/sd1
            tc.tile_pool(name="p_actsh", bufs=1) as p_actsh,  # tags actsh0/1
            tc.tile_pool(name="p_w1", bufs=8) as p_w1,
            tc.tile_pool(name="p_w2", bufs=4) as p_w2,
            tc.tile_pool(name="p_tmp", bufs=3) as p_tmp,
            tc.tile_pool(name="p_actT", bufs=8) as p_actT,
            tc.tile_pool(name="p_y", bufs=3) as p_y,  # tags o_sh / y_e
            tc.tile_pool(name="p_small", bufs=1) as p_small,
            tc.tile_pool(name="ps_all", bufs=4, space="PSUM") as ps_all,  # tag psA
            tc.tile_pool(name="ps_up", bufs=2, space="PSUM") as ps_up,  # tag pu
            tc.tile_pool(name="ps_tr", bufs=2, space="PSUM") as ps_tr,  # tag pt
        ):

            def load_f16(pool, dram_slice, shape, name):
                """Direct DMA fp16 DRAM -> fp16 SBUF tile (no staging/cast)."""
                tl = pool.tile(list(shape), f16, name=name)
                nc.sync.dma_start(tl[:], dram_slice)
                return tl

            # ---- constants / small loads ----
            ident = p_small.tile([P, P], f32, name="ident")
            make_identity(nc, ident[:])

            # ================= shared MLP (TP slice) =================
            act_shT = [p_actsh.tile([P, T], f16, name=f"actsh{m}") for m in range(2)]
            ps_sh = [[None, None], [None, None]]
            for m in range(2):
                for nch in range(2):
                    ps_sh[m][nch] = ps_all.tile([P, 512], f32, name="psA")
            for k in range(KH):
                xt_k = load_f16(p_xt, xt[k * P : (k + 1) * P, :], (P, T), "xt_k")
                su_k = load_f16(
                    p_shupt, shupt[k * P : (k + 1) * P, :], (P, SH_SL), "su_k"
                )
                for m in range(2):
                    for nch in range(2):
                        nc.tensor.matmul(
                            ps_sh[m][nch][:],
                            su_k[:, m * P : (m + 1) * P],
                            xt_k[:, nch * 512 : (nch + 1) * 512],
                            start=(k == 0),
                            stop=(k == KH - 1),
                        )
            for m in range(2):
                for nch in range(2):
                    pp = ps_sh[m][nch]
                    r = p_tmp.tile([P, 512], f32, name="r_sh")
                    nc.scalar.activation(r[:], pp[:], Relu, 0.0, 1.0, 0.0)
                    t2 = p_tmp.tile([P, 512], f32, name="t2_sh")
                    nc.vector.tensor_tensor(
                        out=t2[:], in0=pp[:], in1=r[:], op=mybir.AluOpType.mult
                    )
                    nc.vector.tensor_copy(
                        act_shT[m][:, nch * 512 : (nch + 1) * 512], t2[:]
                    )

            # down: out[t, :] = act_shT.T @ shdownt  (accumulate over 2 k2)
            sd = [
                load_f16(
                    p_shdownt, shdownt[k2 * P : (k2 + 1) * P, :], (P, H), f"sd{k2}"
                )
                for k2 in range(2)
            ]
            for mt in range(T // P):
                o_sh = p_y.tile([P, H], f32, name="o_sh")
                pss = [ps_all.tile([P, 512], f32, name="psA") for h in range(4)]
                for k2 in range(2):
                    for hch in range(4):
                        nc.tensor.matmul(
                            pss[hch][:],
                            act_shT[k2][:, mt * P : (mt + 1) * P],
                            sd[k2][:, hch * 512 : (hch + 1) * 512],
                            start=(k2 == 0),
                            stop=(k2 == 1),
                        )
                for hch in range(4):
                    nc.vector.tensor_copy(
                        o_sh[:, hch * 512 : (hch + 1) * 512], pss[hch][:]
                    )
                nc.sync.dma_start(out[mt * P : (mt + 1) * P, :], o_sh[:])

            # ================= routed experts =================
            for v in range(NV):
                e = v // nslot
                # per-expert gathered tokens, all 16 K-tiles in one DMA:
                # xst[:, v*128:(v+1)*128] = [2048, 128] -> [128, 16*128]
                xs_e = p_xs.tile([P, KH * P], f16, name="xs_e")
                nc.sync.dma_start(
                    xs_e[:].rearrange("p (k c) -> p k c", c=P),
                    xst[:, v * P : (v + 1) * P].rearrange("(k p) c -> p k c", p=P),
                )
                # --- up-projection: psum [128 tok, 512 I] ---
                pu = ps_up.tile([P, I], f32, name="pu")
                for k in range(KH):
                    w1_k = load_f16(
                        p_w1, w1t[e, k * P : (k + 1) * P, :], (P, I), "w1_k"
                    )
                    nc.tensor.matmul(
                        pu[:],
                        xs_e[:, k * P : (k + 1) * P],
                        w1_k[:],
                        start=(k == 0),
                        stop=(k == KH - 1),
                    )
                # --- relu2 ---
                r = p_tmp.tile([P, I], f32, name="r_e")
                nc.scalar.activation(r[:], pu[:], Relu, 0.0, 1.0, 0.0)
                act = p_tmp.tile([P, I], f32, name="act_e")
                nc.vector.tensor_tensor(
                    out=act[:], in0=pu[:], in1=r[:], op=mybir.AluOpType.mult
                )
                # --- transpose act -> actT (4 x [128 I, 128 tok], f16) ---
                actT = []
                for it in range(KI):
                    pt = ps_tr.tile([P, P], f32, name="pt")
                    nc.tensor.transpose(pt[:], act[:, it * P : (it + 1) * P], ident[:])
                    at = p_actT.tile([P, P], f16, name="at")
                    nc.vector.tensor_copy(at[:], pt[:])
                    actT.append(at)
                # --- down-projection: 4 psums [128 tok, 512 H-chunk] ---
                pd = [ps_all.tile([P, 512], f32, name="psA") for h in range(4)]
                for it in range(KI):
                    w2_i = load_f16(
                        p_w2, w2t[e, it * P : (it + 1) * P, :], (P, H), "w2_i"
                    )
                    for hch in range(4):
                        nc.tensor.matmul(
                            pd[hch][:],
                            actT[it][:],
                            w2_i[:, hch * 512 : (hch + 1) * 512],
                            start=(it == 0),
                            stop=(it == KI - 1),
                        )
                y = p_y.tile([P, H], f32, name="y_e")
                for hch in range(4):
                    nc.vector.tensor_copy(y[:, hch * 512 : (hch + 1) * 512], pd[hch][:])
                # --- contiguous write; host scatters rows during unshard ---
                nc.sync.dma_start(yall[v * P : (v + 1) * P, :], y[:])

    nc.compile()
    return nc


def _prepare(inputs):
    """Host gate + dispatch: returns (nc, in_maps) ready for SPMD dispatch."""
    hidden_states = np.asarray(inputs["hidden_states"], dtype=np.float32)
    gate_w = np.asarray(inputs["gate_w"], dtype=np.float32)
    gate_bias = np.asarray(inputs["gate_bias"], dtype=np.float32)
    w1 = np.asarray(inputs["w1"], dtype=np.float32)
    w2 = np.asarray(inputs["w2"], dtype=np.float32)
    shared_up = np.asarray(inputs["shared_up"], dtype=np.float32)
    shared_down = np.asarray(inputs["shared_down"], dtype=np.float32)

    x = hidden_states.reshape(T, H)

    # ---- host gate + dispatch ----
    top_idx, topw = _gate_numpy(x, gate_w, gate_bias)
    sqw = np.sqrt(topw)

    tok_lists = [[] for _ in range(E)]
    scale_lists = [[] for _ in range(E)]
    for kk in range(K):
        for t in range(T):
            e = top_idx[t, kk]
            tok_lists[e].append(t)
            scale_lists[e].append(sqw[t, kk])
    counts = np.array([len(l) for l in tok_lists])
    nslot = max(1, int(np.ceil(counts.max() / P)))

    if nslot not in _PROG_CACHE:
        _PROG_CACHE[nslot] = _build_program(nslot)
    nc = _PROG_CACHE[nslot]

    NV = EL * nslot
    CAP = nslot * P

    xt_np = np.ascontiguousarray(x.T.astype(np.float16))  # [H, T]

    in_maps = []
    for c in range(N_CORES):
        xst_c = np.zeros((H, NV * P), np.float16)
        for j in range(EL):
            e = c * EL + j
            toks = np.array(tok_lists[e], dtype=np.int64)
            scls = np.array(scale_lists[e], dtype=np.float32)
            n = len(toks)
            assert n <= CAP
            if n:
                xs = (x[toks] * scls[:, None]).astype(np.float16)  # [n, H]
                xst_c[:, j * CAP : j * CAP + n] = xs.T
        in_maps.append(
            {
                "xt": xt_np,
                "xst": np.ascontiguousarray(xst_c),
                "w1t": np.ascontiguousarray(
                    w1[c * EL : (c + 1) * EL].transpose(0, 2, 1).astype(np.float16)
                ),
                "w2t": np.ascontiguousarray(
                    w2[c * EL : (c + 1) * EL].transpose(0, 2, 1).astype(np.float16)
                ),
                "shupt": np.ascontiguousarray(
                    shared_up.T[:, c * SH_SL : (c + 1) * SH_SL].astype(np.float16)
                ),
                "shdownt": np.ascontiguousarray(
                    shared_down.T[c * SH_SL : (c + 1) * SH_SL, :].astype(np.float16)
                ),
            }
        )

    return nc, in_maps, tok_lists, nslot


def _combine(results, tok_lists, nslot, out_shape, out_dtype):
    """Host unshard: sum shared partials + scatter-add routed expert rows."""
    CAP = nslot * P
    acc = np.zeros((T, H), np.float32)
    for c in range(N_CORES):
        acc += results[c]["out"]
    for c in range(N_CORES):
        ya = results[c]["yall"]
        for j in range(EL):
            toks = tok_lists[c * EL + j]
            n = len(toks)
            if n:
                acc[toks] += ya[j * CAP : j * CAP + n]
    return acc.reshape(out_shape).astype(out_dtype)


def kernel(**inputs):
    from concourse.bass_utils import run_bass_kernel_spmd

    hidden_states = np.asarray(inputs["hidden_states"], dtype=np.float32)
    nc, in_maps, tok_lists, nslot = _prepare(inputs)
    res = run_bass_kernel_spmd(nc, in_maps, list(range(N_CORES)))
    return _combine(
        res.results, tok_lists, nslot, hidden_states.shape, hidden_states.dtype
    )



# revision 2
# speedup vs baseline: 1.3533x; 1.3533x over previous
"""Self-contained Trainium2 Bass kernel for NemotronH MTP MoE layer.

Expert-parallel over 8 NeuronCores: core c owns experts [8c, 8c+8); the
shared-expert MLP is tensor-parallel sliced (256 of 2048 intermediate dims
per core).  The DeepSeekV3-style gate is computed host-side (tiny), tokens
are dispatched host-side into per-expert 128-token slot blocks with the
combine weight folded in as sqrt(w) (exact: relu^2 is degree-2 homogeneous).

Kernel layout choices are driven by the DMA cost model:
 - every DMA moves >=512B contiguous per descriptor (full 360GB/s rate);
   host pre-blocks all tensors as [128-partition, contiguous-free].
 - outputs are written in f16 (half the bytes of f32; partials are summed
   in f32 on the host, quantization error ~1e-3 relative).
 - the up-projection computes [I-partition, token] tiles directly
   (stationary = w1 chunk, moving = token block), so the down-projection
   needs no PE transposes.
 - loads are issued on the SP sequencer, stores on the Pool (SWDGE)
   sequencer: a store waiting on compute never blocks load dispatch.
"""

import sys

sys.path.insert(0, "/opt/trn_rl_repo")

import numpy as np

# ---- problem constants (hardcoded per contract) ----
B, S, H = 2, 512, 2048
E, G, TOPK_G, K = 64, 8, 4, 6
I = 512
SH_I = 2048
RSF = 2.5
T = B * S  # 1024 tokens
N_CORES = 8
EL = E // N_CORES  # 8 experts per core
SH_SL = SH_I // N_CORES  # 256 shared-intermediate dims per core
P = 128
KH = H // P  # 16 K-tiles over hidden
NI = I // P  # 4 I-planes
NH = H // 512  # 4 H-chunks of 512
TB = T // P  # 8 token blocks

_PROG_CACHE = {}


def _gate_numpy(x, gate_w, gate_bias):
    """noaux_tc gate: sigmoid+bias, group top-2 sum, top-4 groups, top-6."""
    logits = x @ gate_w.T
    scores = 1.0 / (1.0 + np.exp(-logits))
    scores_b = scores + gate_bias
    sb_g = scores_b.reshape(T, G, E // G)
    top2 = np.sort(sb_g, axis=-1)[..., -2:].sum(-1, dtype=np.float32)
    grp_idx = np.argsort(-top2, axis=-1, kind="stable")[:, :TOPK_G]
    grp_mask = np.zeros((T, G), np.float32)
    np.put_along_axis(grp_mask, grp_idx, 1.0, axis=1)
    expert_mask = np.repeat(grp_mask, E // G, axis=-1) > 0
    masked = np.where(expert_mask, scores_b, -np.inf)
    top_idx = np.argsort(-masked, axis=1, kind="stable")[:, :K]
    topw = np.take_along_axis(scores, top_idx, axis=1)
    topw = topw / (topw.sum(-1, keepdims=True, dtype=np.float32) + 1e-20) * RSF
    return top_idx, topw.astype(np.float32)


def _build_program(nslot):
    """Build + compile the SPMD Bass program. nslot = 128-row token-slot
    blocks per expert (1 unless some expert holds >128 tokens)."""
    import concourse.bass as bass
    import concourse.tile as tile
    from concourse import bacc, mybir

    f32 = mybir.dt.float32
    f16 = mybir.dt.float16
    Relu = mybir.ActivationFunctionType.Relu
    Copy = mybir.ActivationFunctionType.Copy

    NV = EL * nslot  # virtual experts (one 128-token slot block each)

    nc = bacc.Bacc("TRN2", target_bir_lowering=False, debug=False, num_devices=N_CORES)

    # blocked DRAM layouts: partition dim second-from-... see _prepare for
    # the exact host-side index formulas.
    xt = nc.dram_tensor("xt", [P, KH, T], f16, kind="ExternalInput").ap()
    xs = nc.dram_tensor("xs", [NV, P, KH, P], f16, kind="ExternalInput").ap()
    w1 = nc.dram_tensor("w1", [EL, P, KH, NI, P], f16, kind="ExternalInput").ap()
    w2 = nc.dram_tensor("w2", [EL, P, NI, H], f16, kind="ExternalInput").ap()
    su = nc.dram_tensor("su", [P, KH, 2, P], f16, kind="ExternalInput").ap()
    sd = nc.dram_tensor("sd", [P, 2, H], f16, kind="ExternalInput").ap()
    out = nc.dram_tensor("out", [T, H], f16, kind="ExternalOutput").ap()
    yall = nc.dram_tensor("yall", [NV * P, H], f16, kind="ExternalOutput").ap()

    with tile.TileContext(nc) as tc:
        with (
            tc.tile_pool(name="p_xt", bufs=1) as p_xt,
            tc.tile_pool(name="p_su", bufs=1) as p_su,
            tc.tile_pool(name="p_sd", bufs=1) as p_sd,
            tc.tile_pool(name="p_ash", bufs=1) as p_ash,
            tc.tile_pool(name="p_xs", bufs=3) as p_xs,
            tc.tile_pool(name="p_w1", bufs=2) as p_w1,
            tc.tile_pool(name="p_w2", bufs=2) as p_w2,
            tc.tile_pool(name="p_r", bufs=2) as p_r,
            tc.tile_pool(name="p_act", bufs=3) as p_act,
            tc.tile_pool(name="p_y", bufs=3) as p_y,
            tc.tile_pool(name="ps_main", bufs=6, space="PSUM") as ps_main,
            tc.tile_pool(name="ps_up", bufs=2, space="PSUM") as ps_up,
        ):
            # ---------------- loads for the shared MLP (SP queue) --------
            xt_t = p_xt.tile([P, KH, T], f16, name="xt")
            nc.sync.dma_start(xt_t[:], xt)
            su_t = p_su.tile([P, KH, 2, P], f16, name="su")
            nc.sync.dma_start(su_t[:], su)
            sd_t = p_sd.tile([P, 2, H], f16, name="sd")
            nc.sync.dma_start(sd_t[:], sd)

            # ---------------- shared up-projection -----------------------
            # psum [128 shI-sub, 512 tok] for (m, nh); contraction over k.
            ps_sh = [[ps_main.tile([P, 512], f32, name="psA") for _ in range(2)]
                     for _ in range(2)]
            for k in range(KH):
                for m in range(2):
                    for nh in range(2):
                        nc.tensor.matmul(
                            ps_sh[m][nh][:],
                            su_t[:, k, m, :],
                            xt_t[:, k, nh * 512 : (nh + 1) * 512],
                            start=(k == 0),
                            stop=(k == KH - 1),
                        )
            # relu^2 -> ash [128, 2(m), 1024 tok] f16
            ash = p_ash.tile([P, 2, T], f16, name="ash")
            for m in range(2):
                for nh in range(2):
                    pp = ps_sh[m][nh]
                    r = p_r.tile([P, 512], f32, name="r_sh")
                    nc.scalar.activation(r[:], pp[:], Relu, 0.0, 1.0, 0.0)
                    nc.vector.tensor_tensor(
                        out=ash[:, m, nh * 512 : (nh + 1) * 512],
                        in0=pp[:],
                        in1=r[:],
                        op=mybir.AluOpType.mult,
                    )

            # ---------------- routed experts + shared-down blocks --------
            def shared_down_block(mt):
                po = [ps_main.tile([P, 512], f32, name="psA") for _ in range(NH)]
                for j in range(2):
                    for hch in range(NH):
                        nc.tensor.matmul(
                            po[hch][:],
                            ash[:, j, mt * P : (mt + 1) * P],
                            sd_t[:, j, hch * 512 : (hch + 1) * 512],
                            start=(j == 0),
                            stop=(j == 1),
                        )
                o_t = p_y.tile([P, H], f16, name="o_t")
                for hch in range(NH):
                    if hch % 2 == 0:
                        nc.vector.tensor_copy(
                            o_t[:, hch * 512 : (hch + 1) * 512], po[hch][:]
                        )
                    else:
                        nc.scalar.activation(
                            o_t[:, hch * 512 : (hch + 1) * 512], po[hch][:], Copy, 0.0
                        )
                nc.gpsimd.dma_start(out[mt * P : (mt + 1) * P, :], o_t[:])

            vcount = 0
            for e in range(EL):
                w1_t = p_w1.tile([P, KH, NI, P], f16, name="w1")
                nc.sync.dma_start(w1_t[:], w1[e])
                w2_t = p_w2.tile([P, NI, H], f16, name="w2")
                nc.sync.dma_start(w2_t[:], w2[e])
                for s_ in range(nslot):
                    v = vcount
                    vcount += 1
                    xs_t = p_xs.tile([P, KH, P], f16, name="xs")
                    nc.sync.dma_start(xs_t[:], xs[v])
                    # up: pu [128 (ci,i) packed free, 128 tok] per ci plane
                    pu = ps_up.tile([P, NI, P], f32, name="pu")
                    for ci in range(NI):
                        for k in range(KH):
                            nc.tensor.matmul(
                                pu[:, ci, :],
                                w1_t[:, k, ci, :],
                                xs_t[:, k, :],
                                start=(k == 0),
                                stop=(k == KH - 1),
                            )
                    # relu^2 -> act [128 i-sub, NI, 128 tok] f16
                    r = p_r.tile([P, NI * P], f32, name="r_e")
                    nc.scalar.activation(
                        r[:], pu[:].rearrange("p a b -> p (a b)"), Relu, 0.0, 1.0, 0.0
                    )
                    act = p_act.tile([P, NI, P], f16, name="act")
                    nc.vector.tensor_tensor(
                        out=act[:].rearrange("p a b -> p (a b)"),
                        in0=pu[:].rearrange("p a b -> p (a b)"),
                        in1=r[:],
                        op=mybir.AluOpType.mult,
                    )
                    # down: pd [128 tok, 512 H-chunk] x4; contraction over ip
                    pd = [ps_main.tile([P, 512], f32, name="psA") for _ in range(NH)]
                    for ip in range(NI):
                        for hch in range(NH):
                            nc.tensor.matmul(
                                pd[hch][:],
                                act[:, ip, :],
                                w2_t[:, ip, hch * 512 : (hch + 1) * 512],
                                start=(ip == 0),
                                stop=(ip == NI - 1),
                            )
                    y = p_y.tile([P, H], f16, name="y_e")
                    for hch in range(NH):
                        if hch % 2 == 0:
                            nc.vector.tensor_copy(
                                y[:, hch * 512 : (hch + 1) * 512], pd[hch][:]
                            )
                        else:
                            nc.scalar.activation(
                                y[:, hch * 512 : (hch + 1) * 512], pd[hch][:], Copy, 0.0
                            )
                    nc.gpsimd.dma_start(yall[v * P : (v + 1) * P, :], y[:])
                    if v < TB:
                        shared_down_block(v)
            for mt in range(vcount, TB):
                shared_down_block(mt)

    nc.compile()
    return nc


def _prepare(inputs):
    """Host gate + dispatch: returns (nc, in_maps) ready for SPMD dispatch."""
    hidden_states = np.asarray(inputs["hidden_states"], dtype=np.float32)
    gate_w = np.asarray(inputs["gate_w"], dtype=np.float32)
    gate_bias = np.asarray(inputs["gate_bias"], dtype=np.float32)
    w1 = np.asarray(inputs["w1"], dtype=np.float32)
    w2 = np.asarray(inputs["w2"], dtype=np.float32)
    shared_up = np.asarray(inputs["shared_up"], dtype=np.float32)
    shared_down = np.asarray(inputs["shared_down"], dtype=np.float32)

    x = hidden_states.reshape(T, H)

    # ---- host gate + dispatch ----
    top_idx, topw = _gate_numpy(x, gate_w, gate_bias)
    sqw = np.sqrt(topw)

    tok_lists = [[] for _ in range(E)]
    scale_lists = [[] for _ in range(E)]
    for kk in range(K):
        for t in range(T):
            e = top_idx[t, kk]
            tok_lists[e].append(t)
            scale_lists[e].append(sqw[t, kk])
    counts = np.array([len(l) for l in tok_lists])
    nslot = max(1, int(np.ceil(counts.max() / P)))

    if nslot not in _PROG_CACHE:
        _PROG_CACHE[nslot] = _build_program(nslot)
    nc = _PROG_CACHE[nslot]

    NV = EL * nslot
    CAP = nslot * P

    # xt blocked: [p, k, t] = x[t, 128k+p]
    xt_b = np.ascontiguousarray(
        x.reshape(T, KH, P).transpose(2, 1, 0).astype(np.float16)
    )

    in_maps = []
    for c in range(N_CORES):
        xs_b = np.zeros((NV, P, KH, P), np.float16)
        for j in range(EL):
            e = c * EL + j
            toks = np.array(tok_lists[e], dtype=np.int64)
            scls = np.array(scale_lists[e], dtype=np.float32)
            n = len(toks)
            assert n <= CAP
            if n:
                xsp = np.zeros((CAP, H), np.float32)
                xsp[:n] = x[toks] * scls[:, None]
                # [v within expert, p, k, c] = xsp[128v+c, 128k+p]
                xs_b[j * nslot : (j + 1) * nslot] = (
                    xsp.reshape(nslot, P, KH, P).transpose(0, 3, 2, 1)
                )
        w1c = w1[c * EL : (c + 1) * EL]  # [EL, I, H]
        w2c = w2[c * EL : (c + 1) * EL]  # [EL, H, I]
        # w1 blocked: [e, p, k, ci, i] = w1[e, 128ci+i, 128k+p]
        w1_b = w1c.reshape(EL, NI, P, KH, P).transpose(0, 4, 3, 1, 2)
        # w2 blocked: [e, p, ip, h] = w2[e, h, 128ip+p]
        w2_b = w2c.reshape(EL, H, NI, P).transpose(0, 3, 2, 1)
        cs = c * SH_SL
        # su blocked: [p, k, cm, i] = shared_up[cs+128cm+i, 128k+p]
        su_b = shared_up[cs : cs + SH_SL].reshape(2, P, KH, P).transpose(3, 2, 0, 1)
        # sd blocked: [p, j, h] = shared_down[h, cs+128j+p]
        sd_b = shared_down[:, cs : cs + SH_SL].reshape(H, 2, P).transpose(2, 1, 0)
        in_maps.append(
            {
                "xt": xt_b,
                "xs": np.ascontiguousarray(xs_b),
                "w1": np.ascontiguousarray(w1_b.astype(np.float16)),
                "w2": np.ascontiguousarray(w2_b.astype(np.float16)),
                "su": np.ascontiguousarray(su_b.astype(np.float16)),
                "sd": np.ascontiguousarray(sd_b.astype(np.float16)),
            }
        )

    return nc, in_maps, tok_lists, nslot


def _combine(results, tok_lists, nslot, out_shape, out_dtype):
    """Host unshard: sum shared partials + scatter-add routed expert rows."""
    CAP = nslot * P
    acc = np.zeros((T, H), np.float32)
    for c in range(N_CORES):
        acc += results[c]["out"].astype(np.float32)
    for c in range(N_CORES):
        ya = results[c]["yall"].astype(np.float32)
        for j in range(EL):
            toks = tok_lists[c * EL + j]
            n = len(toks)
            if n:
                acc[toks] += ya[j * CAP : j * CAP + n]
    return acc.reshape(out_shape).astype(out_dtype)


def kernel(**inputs):
    from concourse.bass_utils import run_bass_kernel_spmd

    hidden_states = np.asarray(inputs["hidden_states"], dtype=np.float32)
    nc, in_maps, tok_lists, nslot = _prepare(inputs)
    res = run_bass_kernel_spmd(nc, in_maps, list(range(N_CORES)))
    return _combine(
        res.results, tok_lists, nslot, hidden_states.shape, hidden_states.dtype
    )


# revision 4
# speedup vs baseline: 1.3905x; 1.0275x over previous
"""Self-contained Trainium2 Bass kernel for NemotronH MTP MoE layer.

Expert-parallel over 8 NeuronCores: core c owns experts [8c, 8c+8); the
shared-expert MLP is tensor-parallel sliced (256 of 2048 intermediate dims
per core).  The DeepSeekV3-style gate is computed host-side (tiny), tokens
are dispatched host-side into per-expert 128-token slot blocks with the
combine weight folded in as sqrt(w) (exact: relu^2 is degree-2 homogeneous).

Kernel layout choices are driven by the DMA cost model:
 - every DMA moves >=512B contiguous per descriptor (full 360GB/s rate);
   host pre-blocks all tensors as [128-partition, contiguous-free].
 - outputs are written in f16 (half the bytes of f32; partials are summed
   in f32 on the host, quantization error ~1e-3 relative).
 - the up-projection computes [I-partition, token] tiles directly
   (stationary = w1 chunk, moving = token block), so the down-projection
   needs no PE transposes.
 - loads are issued on the SP sequencer, stores on the Pool (SWDGE)
   sequencer: a store waiting on compute never blocks load dispatch.
"""

import sys

sys.path.insert(0, "/opt/trn_rl_repo")

import numpy as np

# ---- problem constants (hardcoded per contract) ----
B, S, H = 2, 512, 2048
E, G, TOPK_G, K = 64, 8, 4, 6
I = 512
SH_I = 2048
RSF = 2.5
T = B * S  # 1024 tokens
N_CORES = 8
EL = E // N_CORES  # 8 experts per core
SH_SL = SH_I // N_CORES  # 256 shared-intermediate dims per core
P = 128
KH = H // P  # 16 K-tiles over hidden
NI = I // P  # 4 I-planes
NH = H // 512  # 4 H-chunks of 512
TB = T // P  # 8 token blocks

_PROG_CACHE = {}


def _gate_numpy(x, gate_w, gate_bias):
    """noaux_tc gate: sigmoid+bias, group top-2 sum, top-4 groups, top-6."""
    logits = x @ gate_w.T
    scores = 1.0 / (1.0 + np.exp(-logits))
    scores_b = scores + gate_bias
    sb_g = scores_b.reshape(T, G, E // G)
    top2 = np.sort(sb_g, axis=-1)[..., -2:].sum(-1, dtype=np.float32)
    grp_idx = np.argsort(-top2, axis=-1, kind="stable")[:, :TOPK_G]
    grp_mask = np.zeros((T, G), np.float32)
    np.put_along_axis(grp_mask, grp_idx, 1.0, axis=1)
    expert_mask = np.repeat(grp_mask, E // G, axis=-1) > 0
    masked = np.where(expert_mask, scores_b, -np.inf)
    top_idx = np.argsort(-masked, axis=1, kind="stable")[:, :K]
    topw = np.take_along_axis(scores, top_idx, axis=1)
    topw = topw / (topw.sum(-1, keepdims=True, dtype=np.float32) + 1e-20) * RSF
    return top_idx, topw.astype(np.float32)


def _build_program(nslot):
    """Build + compile the SPMD Bass program. nslot = 128-row token-slot
    blocks per expert (1 unless some expert holds >128 tokens)."""
    import concourse.bass as bass
    import concourse.tile as tile
    from concourse import bacc, mybir

    f32 = mybir.dt.float32
    f16 = mybir.dt.float16
    Relu = mybir.ActivationFunctionType.Relu
    Copy = mybir.ActivationFunctionType.Copy

    NV = EL * nslot  # virtual experts (one 128-token slot block each)

    nc = bacc.Bacc("TRN2", target_bir_lowering=False, debug=False, num_devices=N_CORES)

    # blocked DRAM layouts: partition dim second-from-... see _prepare for
    # the exact host-side index formulas.
    xt = nc.dram_tensor("xt", [P, KH, T], f16, kind="ExternalInput").ap()
    xs = nc.dram_tensor("xs", [NV, P, KH, P], f16, kind="ExternalInput").ap()
    w1 = nc.dram_tensor("w1", [EL, P, KH, NI, P], f16, kind="ExternalInput").ap()
    w2 = nc.dram_tensor("w2", [EL, P, NI, H], f16, kind="ExternalInput").ap()
    su = nc.dram_tensor("su", [P, KH, 2, P], f16, kind="ExternalInput").ap()
    sd = nc.dram_tensor("sd", [P, 2, H], f16, kind="ExternalInput").ap()
    out = nc.dram_tensor("out", [T, H], f16, kind="ExternalOutput").ap()
    yall = nc.dram_tensor("yall", [NV * P, H], f16, kind="ExternalOutput").ap()

    with tile.TileContext(nc) as tc:
        with (
            tc.tile_pool(name="p_xt", bufs=1) as p_xt,
            tc.tile_pool(name="p_su", bufs=1) as p_su,
            tc.tile_pool(name="p_sd", bufs=1) as p_sd,
            tc.tile_pool(name="p_ash", bufs=1) as p_ash,
            tc.tile_pool(name="p_xs", bufs=3) as p_xs,
            tc.tile_pool(name="p_w1", bufs=2) as p_w1,
            tc.tile_pool(name="p_w2", bufs=2) as p_w2,
            tc.tile_pool(name="p_r", bufs=2) as p_r,
            tc.tile_pool(name="p_act", bufs=3) as p_act,
            tc.tile_pool(name="p_y", bufs=3) as p_y,
            tc.tile_pool(name="ps_main", bufs=6, space="PSUM") as ps_main,
            tc.tile_pool(name="ps_up", bufs=2, space="PSUM") as ps_up,
        ):
            # ---------------- loads for the shared MLP (SP queue) --------
            xt_t = p_xt.tile([P, KH, T], f16, name="xt")
            nc.sync.dma_start(xt_t[:], xt)
            su_t = p_su.tile([P, KH, 2, P], f16, name="su")
            nc.sync.dma_start(su_t[:], su)
            sd_t = p_sd.tile([P, 2, H], f16, name="sd")
            nc.sync.dma_start(sd_t[:], sd)

            # ---------------- shared up-projection -----------------------
            # psum [128 shI-sub, 512 tok] for (m, nh); contraction over k.
            ps_sh = [[ps_main.tile([P, 512], f32, name="psA") for _ in range(2)]
                     for _ in range(2)]
            for k in range(KH):
                for m in range(2):
                    for nh in range(2):
                        nc.tensor.matmul(
                            ps_sh[m][nh][:],
                            su_t[:, k, m, :],
                            xt_t[:, k, nh * 512 : (nh + 1) * 512],
                            start=(k == 0),
                            stop=(k == KH - 1),
                        )
            # relu^2 -> ash [128, 2(m), 1024 tok] f16
            ash = p_ash.tile([P, 2, T], f16, name="ash")
            for m in range(2):
                for nh in range(2):
                    pp = ps_sh[m][nh]
                    r = p_r.tile([P, 512], f32, name="r_sh")
                    nc.scalar.activation(r[:], pp[:], Relu, 0.0, 1.0, 0.0)
                    nc.vector.tensor_tensor(
                        out=ash[:, m, nh * 512 : (nh + 1) * 512],
                        in0=pp[:],
                        in1=r[:],
                        op=mybir.AluOpType.mult,
                    )

            # ---------------- routed experts + shared-down blocks --------
            def shared_down_block(mt):
                po = [ps_main.tile([P, 512], f32, name="psA") for _ in range(NH)]
                for j in range(2):
                    for hch in range(NH):
                        nc.tensor.matmul(
                            po[hch][:],
                            ash[:, j, mt * P : (mt + 1) * P],
                            sd_t[:, j, hch * 512 : (hch + 1) * 512],
                            start=(j == 0),
                            stop=(j == 1),
                        )
                o_t = p_y.tile([P, H], f16, name="o_t")
                for hch in range(NH):
                    if hch % 2 == 0:
                        nc.vector.tensor_copy(
                            o_t[:, hch * 512 : (hch + 1) * 512], po[hch][:]
                        )
                    else:
                        nc.scalar.activation(
                            o_t[:, hch * 512 : (hch + 1) * 512], po[hch][:], Copy, 0.0
                        )
                nc.gpsimd.dma_start(out[mt * P : (mt + 1) * P, :], o_t[:])

            vcount = 0
            for e in range(EL):
                xs_ts = []
                for s_ in range(nslot):
                    xs_t = p_xs.tile([P, KH, P], f16, name="xs")
                    nc.sync.dma_start(xs_t[:], xs[e * nslot + s_])
                    xs_ts.append(xs_t)
                w1_t = p_w1.tile([P, KH, NI, P], f16, name="w1")
                nc.sync.dma_start(w1_t[:], w1[e])
                w2_t = p_w2.tile([P, NI, H], f16, name="w2")
                nc.sync.dma_start(w2_t[:], w2[e])
                for s_ in range(nslot):
                    v = vcount
                    vcount += 1
                    xs_t = xs_ts[s_]
                    # up: pu [128 (ci,i) packed free, 128 tok] per ci plane
                    pu = ps_up.tile([P, NI, P], f32, name="pu")
                    for ci in range(NI):
                        for k in range(KH):
                            nc.tensor.matmul(
                                pu[:, ci, :],
                                w1_t[:, k, ci, :],
                                xs_t[:, k, :],
                                start=(k == 0),
                                stop=(k == KH - 1),
                            )
                    # relu^2 -> act [128 i-sub, NI, 128 tok] f16
                    r = p_r.tile([P, NI * P], f32, name="r_e")
                    nc.scalar.activation(
                        r[:], pu[:].rearrange("p a b -> p (a b)"), Relu, 0.0, 1.0, 0.0
                    )
                    act = p_act.tile([P, NI, P], f16, name="act")
                    nc.vector.tensor_tensor(
                        out=act[:].rearrange("p a b -> p (a b)"),
                        in0=pu[:].rearrange("p a b -> p (a b)"),
                        in1=r[:],
                        op=mybir.AluOpType.mult,
                    )
                    # down: pd [128 tok, 512 H-chunk] x4; contraction over ip
                    pd = [ps_main.tile([P, 512], f32, name="psA") for _ in range(NH)]
                    for ip in range(NI):
                        for hch in range(NH):
                            nc.tensor.matmul(
                                pd[hch][:],
                                act[:, ip, :],
                                w2_t[:, ip, hch * 512 : (hch + 1) * 512],
                                start=(ip == 0),
                                stop=(ip == NI - 1),
                            )
                    y = p_y.tile([P, H], f16, name="y_e")
                    for hch in range(NH):
                        if hch % 2 == 0:
                            nc.vector.tensor_copy(
                                y[:, hch * 512 : (hch + 1) * 512], pd[hch][:]
                            )
                        else:
                            nc.scalar.activation(
                                y[:, hch * 512 : (hch + 1) * 512], pd[hch][:], Copy, 0.0
                            )
                    nc.gpsimd.dma_start(yall[v * P : (v + 1) * P, :], y[:])
            # shared-down blocks last: their stores pack the DMA tail while
            # the final expert's compute drains.
            for mt in range(TB):
                shared_down_block(mt)

    nc.compile()
    return nc


def _prepare(inputs):
    """Host gate + dispatch: returns (nc, in_maps) ready for SPMD dispatch."""
    hidden_states = np.asarray(inputs["hidden_states"], dtype=np.float32)
    gate_w = np.asarray(inputs["gate_w"], dtype=np.float32)
    gate_bias = np.asarray(inputs["gate_bias"], dtype=np.float32)
    w1 = np.asarray(inputs["w1"], dtype=np.float32)
    w2 = np.asarray(inputs["w2"], dtype=np.float32)
    shared_up = np.asarray(inputs["shared_up"], dtype=np.float32)
    shared_down = np.asarray(inputs["shared_down"], dtype=np.float32)

    x = hidden_states.reshape(T, H)

    # ---- host gate + dispatch ----
    top_idx, topw = _gate_numpy(x, gate_w, gate_bias)
    sqw = np.sqrt(topw)

    tok_lists = [[] for _ in range(E)]
    scale_lists = [[] for _ in range(E)]
    for kk in range(K):
        for t in range(T):
            e = top_idx[t, kk]
            tok_lists[e].append(t)
            scale_lists[e].append(sqw[t, kk])
    counts = np.array([len(l) for l in tok_lists])
    nslot = max(1, int(np.ceil(counts.max() / P)))

    if nslot not in _PROG_CACHE:
        _PROG_CACHE[nslot] = _build_program(nslot)
    nc = _PROG_CACHE[nslot]

    NV = EL * nslot
    CAP = nslot * P

    # xt blocked: [p, k, t] = x[t, 128k+p]
    xt_b = np.ascontiguousarray(
        x.reshape(T, KH, P).transpose(2, 1, 0).astype(np.float16)
    )

    in_maps = []
    for c in range(N_CORES):
        xs_b = np.zeros((NV, P, KH, P), np.float16)
        for j in range(EL):
            e = c * EL + j
            toks = np.array(tok_lists[e], dtype=np.int64)
            scls = np.array(scale_lists[e], dtype=np.float32)
            n = len(toks)
            assert n <= CAP
            if n:
                xsp = np.zeros((CAP, H), np.float32)
                xsp[:n] = x[toks] * scls[:, None]
                # [v within expert, p, k, c] = xsp[128v+c, 128k+p]
                xs_b[j * nslot : (j + 1) * nslot] = (
                    xsp.reshape(nslot, P, KH, P).transpose(0, 3, 2, 1)
                )
        w1c = w1[c * EL : (c + 1) * EL]  # [EL, I, H]
        w2c = w2[c * EL : (c + 1) * EL]  # [EL, H, I]
        # w1 blocked: [e, p, k, ci, i] = w1[e, 128ci+i, 128k+p]
        w1_b = w1c.reshape(EL, NI, P, KH, P).transpose(0, 4, 3, 1, 2)
        # w2 blocked: [e, p, ip, h] = w2[e, h, 128ip+p]
        w2_b = w2c.reshape(EL, H, NI, P).transpose(0, 3, 2, 1)
        cs = c * SH_SL
        # su blocked: [p, k, cm, i] = shared_up[cs+128cm+i, 128k+p]
        su_b = shared_up[cs : cs + SH_SL].reshape(2, P, KH, P).transpose(3, 2, 0, 1)
        # sd blocked: [p, j, h] = shared_down[h, cs+128j+p]
        sd_b = shared_down[:, cs : cs + SH_SL].reshape(H, 2, P).transpose(2, 1, 0)
        in_maps.append(
            {
                "xt": xt_b,
                "xs": np.ascontiguousarray(xs_b),
                "w1": np.ascontiguousarray(w1_b.astype(np.float16)),
                "w2": np.ascontiguousarray(w2_b.astype(np.float16)),
                "su": np.ascontiguousarray(su_b.astype(np.float16)),
                "sd": np.ascontiguousarray(sd_b.astype(np.float16)),
            }
        )

    return nc, in_maps, tok_lists, nslot


def _combine(results, tok_lists, nslot, out_shape, out_dtype):
    """Host unshard: sum shared partials + scatter-add routed expert rows."""
    CAP = nslot * P
    acc = np.zeros((T, H), np.float32)
    for c in range(N_CORES):
        acc += results[c]["out"].astype(np.float32)
    for c in range(N_CORES):
        ya = results[c]["yall"].astype(np.float32)
        for j in range(EL):
            toks = tok_lists[c * EL + j]
            n = len(toks)
            if n:
                acc[toks] += ya[j * CAP : j * CAP + n]
    return acc.reshape(out_shape).astype(out_dtype)


def kernel(**inputs):
    from concourse.bass_utils import run_bass_kernel_spmd

    hidden_states = np.asarray(inputs["hidden_states"], dtype=np.float32)
    nc, in_maps, tok_lists, nslot = _prepare(inputs)
    res = run_bass_kernel_spmd(nc, in_maps, list(range(N_CORES)))
    return _combine(
        res.results, tok_lists, nslot, hidden_states.shape, hidden_states.dtype
    )


# revision 6
# speedup vs baseline: 1.4036x; 1.0094x over previous
"""Self-contained Trainium2 Bass kernel for NemotronH MTP MoE layer.

Expert-parallel over 8 NeuronCores: core c owns experts [8c, 8c+8); the
shared-expert MLP is tensor-parallel sliced (256 of 2048 intermediate dims
per core).  The DeepSeekV3-style gate is computed host-side (tiny), tokens
are dispatched host-side into per-expert 128-token slot blocks with the
combine weight folded in as sqrt(w) (exact: relu^2 is degree-2 homogeneous).

Kernel layout choices are driven by the DMA cost model:
 - every DMA moves >=512B contiguous per descriptor (full 360GB/s rate);
   host pre-blocks all tensors as [128-partition, contiguous-free].
 - outputs are written in f16 (half the bytes of f32; partials are summed
   in f32 on the host, quantization error ~1e-3 relative).
 - the up-projection computes [I-partition, token] tiles directly
   (stationary = w1 chunk, moving = token block), so the down-projection
   needs no PE transposes.
 - loads are issued on the SP sequencer, stores on the Pool (SWDGE)
   sequencer: a store waiting on compute never blocks load dispatch.
"""

import sys

sys.path.insert(0, "/opt/trn_rl_repo")

import numpy as np

# ---- problem constants (hardcoded per contract) ----
B, S, H = 2, 512, 2048
E, G, TOPK_G, K = 64, 8, 4, 6
I = 512
SH_I = 2048
RSF = 2.5
T = B * S  # 1024 tokens
N_CORES = 8
EL = E // N_CORES  # 8 experts per core
SH_SL = SH_I // N_CORES  # 256 shared-intermediate dims per core
P = 128
KH = H // P  # 16 K-tiles over hidden
NI = I // P  # 4 I-planes
NH = H // 512  # 4 H-chunks of 512
TB = T // P  # 8 token blocks

_PROG_CACHE = {}


def _gate_numpy(x, gate_w, gate_bias):
    """noaux_tc gate: sigmoid+bias, group top-2 sum, top-4 groups, top-6."""
    logits = x @ gate_w.T
    scores = 1.0 / (1.0 + np.exp(-logits))
    scores_b = scores + gate_bias
    sb_g = scores_b.reshape(T, G, E // G)
    top2 = np.sort(sb_g, axis=-1)[..., -2:].sum(-1, dtype=np.float32)
    grp_idx = np.argsort(-top2, axis=-1, kind="stable")[:, :TOPK_G]
    grp_mask = np.zeros((T, G), np.float32)
    np.put_along_axis(grp_mask, grp_idx, 1.0, axis=1)
    expert_mask = np.repeat(grp_mask, E // G, axis=-1) > 0
    masked = np.where(expert_mask, scores_b, -np.inf)
    top_idx = np.argsort(-masked, axis=1, kind="stable")[:, :K]
    topw = np.take_along_axis(scores, top_idx, axis=1)
    topw = topw / (topw.sum(-1, keepdims=True, dtype=np.float32) + 1e-20) * RSF
    return top_idx, topw.astype(np.float32)


def _build_program(nslot):
    """Build + compile the SPMD Bass program. nslot = 128-row token-slot
    blocks per expert (1 unless some expert holds >128 tokens)."""
    import concourse.bass as bass
    import concourse.tile as tile
    from concourse import bacc, mybir

    f32 = mybir.dt.float32
    f16 = mybir.dt.float16
    Relu = mybir.ActivationFunctionType.Relu
    Copy = mybir.ActivationFunctionType.Copy

    NV = EL * nslot  # virtual experts (one 128-token slot block each)

    nc = bacc.Bacc("TRN2", target_bir_lowering=False, debug=False, num_devices=N_CORES)

    # blocked DRAM layouts: partition dim second-from-... see _prepare for
    # the exact host-side index formulas.
    xt = nc.dram_tensor("xt", [P, KH, T], f16, kind="ExternalInput").ap()
    xs = nc.dram_tensor("xs", [NV, P, KH, P], f16, kind="ExternalInput").ap()
    w1 = nc.dram_tensor("w1", [EL, P, KH, NI, P], f16, kind="ExternalInput").ap()
    w2 = nc.dram_tensor("w2", [EL, P, NI, H], f16, kind="ExternalInput").ap()
    su = nc.dram_tensor("su", [P, KH, 2, P], f16, kind="ExternalInput").ap()
    sd = nc.dram_tensor("sd", [P, 2, H], f16, kind="ExternalInput").ap()
    out = nc.dram_tensor("out", [T, H], f16, kind="ExternalOutput").ap()
    yall = nc.dram_tensor("yall", [NV * P, H], f16, kind="ExternalOutput").ap()

    with tile.TileContext(nc) as tc:
        with (
            tc.tile_pool(name="p_xt", bufs=1) as p_xt,
            tc.tile_pool(name="p_su", bufs=1) as p_su,
            tc.tile_pool(name="p_sd", bufs=1) as p_sd,
            tc.tile_pool(name="p_ash", bufs=1) as p_ash,
            tc.tile_pool(name="p_xs", bufs=4) as p_xs,
            tc.tile_pool(name="p_w1", bufs=3) as p_w1,
            tc.tile_pool(name="p_w2", bufs=3) as p_w2,
            tc.tile_pool(name="p_r", bufs=2) as p_r,
            tc.tile_pool(name="p_act", bufs=3) as p_act,
            tc.tile_pool(name="p_y", bufs=3) as p_y,
            tc.tile_pool(name="ps_main", bufs=6, space="PSUM") as ps_main,
            tc.tile_pool(name="ps_up", bufs=2, space="PSUM") as ps_up,
        ):
            # ---------------- loads for the shared MLP (SP queue) --------
            xt_t = p_xt.tile([P, KH, T], f16, name="xt")
            nc.sync.dma_start(xt_t[:], xt)
            su_t = p_su.tile([P, KH, 2, P], f16, name="su")
            nc.sync.dma_start(su_t[:], su)
            sd_t = p_sd.tile([P, 2, H], f16, name="sd")
            nc.sync.dma_start(sd_t[:], sd)

            # ---------------- shared up-projection -----------------------
            # psum [128 shI-sub, 512 tok] for (m, nh); contraction over k.
            ps_sh = [[ps_main.tile([P, 512], f32, name="psA") for _ in range(2)]
                     for _ in range(2)]
            for k in range(KH):
                for m in range(2):
                    for nh in range(2):
                        nc.tensor.matmul(
                            ps_sh[m][nh][:],
                            su_t[:, k, m, :],
                            xt_t[:, k, nh * 512 : (nh + 1) * 512],
                            start=(k == 0),
                            stop=(k == KH - 1),
                        )
            # relu^2 -> ash [128, 2(m), 1024 tok] f16
            ash = p_ash.tile([P, 2, T], f16, name="ash")
            for m in range(2):
                for nh in range(2):
                    pp = ps_sh[m][nh]
                    r = p_r.tile([P, 512], f32, name="r_sh")
                    nc.scalar.activation(r[:], pp[:], Relu, 0.0, 1.0, 0.0)
                    nc.vector.tensor_tensor(
                        out=ash[:, m, nh * 512 : (nh + 1) * 512],
                        in0=pp[:],
                        in1=r[:],
                        op=mybir.AluOpType.mult,
                    )

            # ---------------- routed experts + shared-down blocks --------
            def shared_down_block(mt):
                po = [ps_main.tile([P, 512], f32, name="psA") for _ in range(NH)]
                for j in range(2):
                    for hch in range(NH):
                        nc.tensor.matmul(
                            po[hch][:],
                            ash[:, j, mt * P : (mt + 1) * P],
                            sd_t[:, j, hch * 512 : (hch + 1) * 512],
                            start=(j == 0),
                            stop=(j == 1),
                        )
                o_t = p_y.tile([P, H], f16, name="o_t")
                for hch in range(NH):
                    if hch % 2 == 0:
                        nc.vector.tensor_copy(
                            o_t[:, hch * 512 : (hch + 1) * 512], po[hch][:]
                        )
                    else:
                        nc.scalar.activation(
                            o_t[:, hch * 512 : (hch + 1) * 512], po[hch][:], Copy, 0.0
                        )
                nc.gpsimd.dma_start(out[mt * P : (mt + 1) * P, :], o_t[:])

            def expert_down(st):
                """down-projection + output copies + store for a staged expert."""
                v, act, w2_t = st
                pd = [ps_main.tile([P, 512], f32, name="psA") for _ in range(NH)]
                for ip in range(NI):
                    for hch in range(NH):
                        nc.tensor.matmul(
                            pd[hch][:],
                            act[:, ip, :],
                            w2_t[:, ip, hch * 512 : (hch + 1) * 512],
                            start=(ip == 0),
                            stop=(ip == NI - 1),
                        )
                y = p_y.tile([P, H], f16, name="y_e")
                for hch in range(NH):
                    if hch % 2 == 0:
                        nc.vector.tensor_copy(
                            y[:, hch * 512 : (hch + 1) * 512], pd[hch][:]
                        )
                    else:
                        nc.scalar.activation(
                            y[:, hch * 512 : (hch + 1) * 512], pd[hch][:], Copy, 0.0
                        )
                nc.gpsimd.dma_start(yall[v * P : (v + 1) * P, :], y[:])

            # software-pipelined: expert e's down-projection issues after
            # expert e+1's up-projection, so the PE never waits on the
            # relu^2 (Act+DVE) stage and stays ramped.
            vcount = 0
            staged = None
            for e in range(EL):
                xs_ts = []
                for s_ in range(nslot):
                    xs_t = p_xs.tile([P, KH, P], f16, name="xs")
                    nc.sync.dma_start(xs_t[:], xs[e * nslot + s_])
                    xs_ts.append(xs_t)
                w1_t = p_w1.tile([P, KH, NI, P], f16, name="w1")
                nc.sync.dma_start(w1_t[:], w1[e])
                w2_t = p_w2.tile([P, NI, H], f16, name="w2")
                nc.sync.dma_start(w2_t[:], w2[e])
                for s_ in range(nslot):
                    v = vcount
                    vcount += 1
                    xs_t = xs_ts[s_]
                    # up: pu [128 i-sub, NI, 128 tok]
                    pu = ps_up.tile([P, NI, P], f32, name="pu")
                    for ci in range(NI):
                        for k in range(KH):
                            nc.tensor.matmul(
                                pu[:, ci, :],
                                w1_t[:, k, ci, :],
                                xs_t[:, k, :],
                                start=(k == 0),
                                stop=(k == KH - 1),
                            )
                    # relu^2 -> act [128 i-sub, NI, 128 tok] f16
                    r = p_r.tile([P, NI * P], f32, name="r_e")
                    nc.scalar.activation(
                        r[:], pu[:].rearrange("p a b -> p (a b)"), Relu, 0.0, 1.0, 0.0
                    )
                    act = p_act.tile([P, NI, P], f16, name="act")
                    nc.vector.tensor_tensor(
                        out=act[:].rearrange("p a b -> p (a b)"),
                        in0=pu[:].rearrange("p a b -> p (a b)"),
                        in1=r[:],
                        op=mybir.AluOpType.mult,
                    )
                    if staged is not None:
                        expert_down(staged)
                    staged = (v, act, w2_t)
            if staged is not None:
                expert_down(staged)
            # shared-down blocks last: their stores pack the DMA tail while
            # the final expert's compute drains.
            for mt in range(TB):
                shared_down_block(mt)

    nc.compile()
    return nc


def _prepare(inputs):
    """Host gate + dispatch: returns (nc, in_maps) ready for SPMD dispatch."""
    hidden_states = np.asarray(inputs["hidden_states"], dtype=np.float32)
    gate_w = np.asarray(inputs["gate_w"], dtype=np.float32)
    gate_bias = np.asarray(inputs["gate_bias"], dtype=np.float32)
    w1 = np.asarray(inputs["w1"], dtype=np.float32)
    w2 = np.asarray(inputs["w2"], dtype=np.float32)
    shared_up = np.asarray(inputs["shared_up"], dtype=np.float32)
    shared_down = np.asarray(inputs["shared_down"], dtype=np.float32)

    x = hidden_states.reshape(T, H)

    # ---- host gate + dispatch ----
    top_idx, topw = _gate_numpy(x, gate_w, gate_bias)
    sqw = np.sqrt(topw)

    tok_lists = [[] for _ in range(E)]
    scale_lists = [[] for _ in range(E)]
    for kk in range(K):
        for t in range(T):
            e = top_idx[t, kk]
            tok_lists[e].append(t)
            scale_lists[e].append(sqw[t, kk])
    counts = np.array([len(l) for l in tok_lists])
    nslot = max(1, int(np.ceil(counts.max() / P)))

    if nslot not in _PROG_CACHE:
        _PROG_CACHE[nslot] = _build_program(nslot)
    nc = _PROG_CACHE[nslot]

    NV = EL * nslot
    CAP = nslot * P

    # xt blocked: [p, k, t] = x[t, 128k+p]
    xt_b = np.ascontiguousarray(
        x.reshape(T, KH, P).transpose(2, 1, 0).astype(np.float16)
    )

    in_maps = []
    for c in range(N_CORES):
        xs_b = np.zeros((NV, P, KH, P), np.float16)
        for j in range(EL):
            e = c * EL + j
            toks = np.array(tok_lists[e], dtype=np.int64)
            scls = np.array(scale_lists[e], dtype=np.float32)
            n = len(toks)
            assert n <= CAP
            if n:
                xsp = np.zeros((CAP, H), np.float32)
                xsp[:n] = x[toks] * scls[:, None]
                # [v within expert, p, k, c] = xsp[128v+c, 128k+p]
                xs_b[j * nslot : (j + 1) * nslot] = (
                    xsp.reshape(nslot, P, KH, P).transpose(0, 3, 2, 1)
                )
        w1c = w1[c * EL : (c + 1) * EL]  # [EL, I, H]
        w2c = w2[c * EL : (c + 1) * EL]  # [EL, H, I]
        # w1 blocked: [e, p, k, ci, i] = w1[e, 128ci+i, 128k+p]
        w1_b = w1c.reshape(EL, NI, P, KH, P).transpose(0, 4, 3, 1, 2)
        # w2 blocked: [e, p, ip, h] = w2[e, h, 128ip+p]
        w2_b = w2c.reshape(EL, H, NI, P).transpose(0, 3, 2, 1)
        cs = c * SH_SL
        # su blocked: [p, k, cm, i] = shared_up[cs+128cm+i, 128k+p]
        su_b = shared_up[cs : cs + SH_SL].reshape(2, P, KH, P).transpose(3, 2, 0, 1)
        # sd blocked: [p, j, h] = shared_down[h, cs+128j+p]
        sd_b = shared_down[:, cs : cs + SH_SL].reshape(H, 2, P).transpose(2, 1, 0)
        in_maps.append(
            {
                "xt": xt_b,
                "xs": np.ascontiguousarray(xs_b),
                "w1": np.ascontiguousarray(w1_b.astype(np.float16)),
                "w2": np.ascontiguousarray(w2_b.astype(np.float16)),
                "su": np.ascontiguousarray(su_b.astype(np.float16)),
                "sd": np.ascontiguousarray(sd_b.astype(np.float16)),
            }
        )

    return nc, in_maps, tok_lists, nslot


def _combine(results, tok_lists, nslot, out_shape, out_dtype):
    """Host unshard: sum shared partials + scatter-add routed expert rows."""
    CAP = nslot * P
    acc = np.zeros((T, H), np.float32)
    for c in range(N_CORES):
        acc += results[c]["out"].astype(np.float32)
    for c in range(N_CORES):
        ya = results[c]["yall"].astype(np.float32)
        for j in range(EL):
            toks = tok_lists[c * EL + j]
            n = len(toks)
            if n:
                acc[toks] += ya[j * CAP : j * CAP + n]
    return acc.reshape(out_shape).astype(out_dtype)


def kernel(**inputs):
    from concourse.bass_utils import run_bass_kernel_spmd

    hidden_states = np.asarray(inputs["hidden_states"], dtype=np.float32)
    nc, in_maps, tok_lists, nslot = _prepare(inputs)
    res = run_bass_kernel_spmd(nc, in_maps, list(range(N_CORES)))
    return _combine(
        res.results, tok_lists, nslot, hidden_states.shape, hidden_states.dtype
    )


# revision 7
# speedup vs baseline: 1.4709x; 1.0480x over previous
"""Self-contained Trainium2 Bass kernel for NemotronH MTP MoE layer.

Expert-parallel over 8 NeuronCores: core c owns experts [8c, 8c+8).  The
shared-expert MLP is split 2x4 (2 token halves x 4 slices of 512 of the
2048 intermediate dims): splitting tokens halves both the x load and the
partial-output store of the shared path at the cost of +2MB of shared
weights -- net DMA win, and only 4 partials sum per token.

The DeepSeekV3-style gate runs host-side (tiny); tokens are dispatched
host-side into per-expert 128-token slot blocks with the combine weight
folded in as sqrt(w) (exact: relu^2 is degree-2 homogeneous).

Kernel layout choices are driven by the DMA cost model:
 - every DMA moves >=512B contiguous per descriptor (full-rate);
   host pre-blocks all tensors as [128-partition, contiguous-free].
 - dispatched tokens (xs) are f8e3m4 (x2 scale, compensated in the relu
   scale); outputs are f16; weights stay f16 for accuracy.
 - the up-projection computes [I-partition, token] tiles directly
   (stationary = w1 chunk, moving = token block), so the down-projection
   needs no PE transposes.
 - loads are issued on the SP sequencer, stores on the Pool (SWDGE)
   sequencer: a store waiting on compute never blocks load dispatch.
 - expert pipeline is software-pipelined (down-proj of expert e issues
   after up-proj of e+1) to keep the PE ramped.
"""

import sys

sys.path.insert(0, "/opt/trn_rl_repo")

import numpy as np

# ---- problem constants (hardcoded per contract) ----
B, S, H = 2, 512, 2048
E, G, TOPK_G, K = 64, 8, 4, 6
I = 512
SH_I = 2048
RSF = 2.5
T = B * S  # 1024 tokens
N_CORES = 8
EL = E // N_CORES  # 8 experts per core
P = 128
KH = H // P  # 16 K-tiles over hidden
NI = I // P  # 4 I-planes
NH = H // 512  # 4 H-chunks of 512
# shared-MLP 2x4 split
T2 = T // 2  # tokens per core for the shared path
SQ = SH_I // 4  # shared-intermediate dims per core (512 = 4 planes)
NSQ = SQ // P  # 4
TB2 = T2 // P  # 4 shared token blocks

XS_S = 2.0  # host scale folded into f8e3m4 dispatched tokens

_PROG_CACHE = {}


def _gate_numpy(x, gate_w, gate_bias):
    """noaux_tc gate: sigmoid+bias, group top-2 sum, top-4 groups, top-6."""
    logits = x @ gate_w.T
    scores = 1.0 / (1.0 + np.exp(-logits))
    scores_b = scores + gate_bias
    sb_g = scores_b.reshape(T, G, E // G)
    top2 = np.sort(sb_g, axis=-1)[..., -2:].sum(-1, dtype=np.float32)
    grp_idx = np.argsort(-top2, axis=-1, kind="stable")[:, :TOPK_G]
    grp_mask = np.zeros((T, G), np.float32)
    np.put_along_axis(grp_mask, grp_idx, 1.0, axis=1)
    expert_mask = np.repeat(grp_mask, E // G, axis=-1) > 0
    masked = np.where(expert_mask, scores_b, -np.inf)
    top_idx = np.argsort(-masked, axis=1, kind="stable")[:, :K]
    topw = np.take_along_axis(scores, top_idx, axis=1)
    topw = topw / (topw.sum(-1, keepdims=True, dtype=np.float32) + 1e-20) * RSF
    return top_idx, topw.astype(np.float32)


def _build_program(nslot):
    """Build + compile the SPMD Bass program. nslot = 128-row token-slot
    blocks per expert (1 unless some expert holds >128 tokens)."""
    import concourse.bass as bass
    import concourse.tile as tile
    from concourse import bacc, mybir

    f32 = mybir.dt.float32
    f16 = mybir.dt.float16
    f8e3 = mybir.dt.float8e3
    Relu = mybir.ActivationFunctionType.Relu
    Copy = mybir.ActivationFunctionType.Copy

    NV = EL * nslot  # virtual experts (one 128-token slot block each)

    nc = bacc.Bacc("TRN2", target_bir_lowering=False, debug=False, num_devices=N_CORES)

    # blocked DRAM layouts; see _prepare for the host-side index formulas.
    xt = nc.dram_tensor("xt", [P, KH, T2], f16, kind="ExternalInput").ap()
    xs = nc.dram_tensor("xs", [NV, P, KH, P], f8e3, kind="ExternalInput").ap()
    w1 = nc.dram_tensor("w1", [EL, P, KH, NI, P], f16, kind="ExternalInput").ap()
    w2 = nc.dram_tensor("w2", [EL, P, NI, H], f16, kind="ExternalInput").ap()
    su = nc.dram_tensor("su", [P, KH, NSQ, P], f16, kind="ExternalInput").ap()
    sd = nc.dram_tensor("sd", [P, NSQ, H], f16, kind="ExternalInput").ap()
    out = nc.dram_tensor("out", [T2, H], f16, kind="ExternalOutput").ap()
    yall = nc.dram_tensor("yall", [NV * P, H], f16, kind="ExternalOutput").ap()

    with tile.TileContext(nc) as tc:
        with (
            tc.tile_pool(name="p_xt", bufs=1) as p_xt,
            tc.tile_pool(name="p_su", bufs=1) as p_su,
            tc.tile_pool(name="p_sd", bufs=1) as p_sd,
            tc.tile_pool(name="p_ash", bufs=1) as p_ash,
            tc.tile_pool(name="p_xs", bufs=4) as p_xs,
            tc.tile_pool(name="p_w1", bufs=3) as p_w1,
            tc.tile_pool(name="p_w2", bufs=3) as p_w2,
            tc.tile_pool(name="p_r", bufs=2) as p_r,
            tc.tile_pool(name="p_act", bufs=3) as p_act,
            tc.tile_pool(name="p_y", bufs=3) as p_y,
            tc.tile_pool(name="ps_main", bufs=6, space="PSUM") as ps_main,
            tc.tile_pool(name="ps_up", bufs=2, space="PSUM") as ps_up,
        ):
            # ---------------- loads for the shared MLP (SP queue) --------
            xt_t = p_xt.tile([P, KH, T2], f16, name="xt")
            nc.sync.dma_start(xt_t[:], xt)
            su_t = p_su.tile([P, KH, NSQ, P], f16, name="su")
            nc.sync.dma_start(su_t[:], su)
            sd_t = p_sd.tile([P, NSQ, H], f16, name="sd")
            nc.sync.dma_start(sd_t[:], sd)

            # ---------------- shared up-projection -----------------------
            # psum [128 shI-sub, 512 tok] per m-plane; contraction over k.
            ps_sh = [ps_main.tile([P, T2], f32, name="psA") for _ in range(NSQ)]
            for k in range(KH):
                for m in range(NSQ):
                    nc.tensor.matmul(
                        ps_sh[m][:],
                        su_t[:, k, m, :],
                        xt_t[:, k, :],
                        start=(k == 0),
                        stop=(k == KH - 1),
                    )
            # relu^2 -> ash [128, NSQ(m), 512 tok] f16
            ash = p_ash.tile([P, NSQ, T2], f16, name="ash")
            for m in range(NSQ):
                pp = ps_sh[m]
                r = p_r.tile([P, T2], f32, name="r_sh")
                nc.scalar.activation(r[:], pp[:], Relu, 0.0, 1.0, 0.0)
                nc.vector.tensor_tensor(
                    out=ash[:, m, :], in0=pp[:], in1=r[:], op=mybir.AluOpType.mult
                )

            # ---------------- routed experts (software-pipelined) --------
            def expert_down(st):
                """down-projection + output copies + store for a staged expert."""
                v, act, w2_t = st
                pd = [ps_main.tile([P, 512], f32, name="psA") for _ in range(NH)]
                for ip in range(NI):
                    for hch in range(NH):
                        nc.tensor.matmul(
                            pd[hch][:],
                            act[:, ip, :],
                            w2_t[:, ip, hch * 512 : (hch + 1) * 512],
                            start=(ip == 0),
                            stop=(ip == NI - 1),
                        )
                y = p_y.tile([P, H], f16, name="y_e")
                for hch in range(NH):
                    if hch % 2 == 0:
                        nc.vector.tensor_copy(
                            y[:, hch * 512 : (hch + 1) * 512], pd[hch][:]
                        )
                    else:
                        nc.scalar.activation(
                            y[:, hch * 512 : (hch + 1) * 512], pd[hch][:], Copy, 0.0
                        )
                nc.gpsimd.dma_start(yall[v * P : (v + 1) * P, :], y[:])

            vcount = 0
            staged = None
            for e in range(EL):
                xs_ts = []
                for s_ in range(nslot):
                    xs_t = p_xs.tile([P, KH, P], f8e3, name="xs")
                    nc.sync.dma_start(xs_t[:], xs[e * nslot + s_])
                    xs_ts.append(xs_t)
                w1_t = p_w1.tile([P, KH, NI, P], f16, name="w1")
                nc.sync.dma_start(w1_t[:], w1[e])
                w2_t = p_w2.tile([P, NI, H], f16, name="w2")
                nc.sync.dma_start(w2_t[:], w2[e])
                for s_ in range(nslot):
                    v = vcount
                    vcount += 1
                    xs_t = xs_ts[s_]
                    # up: pu [128 i-sub, NI, 128 tok]
                    pu = ps_up.tile([P, NI, P], f32, name="pu")
                    for ci in range(NI):
                        for k in range(KH):
                            nc.tensor.matmul(
                                pu[:, ci, :],
                                w1_t[:, k, ci, :],
                                xs_t[:, k, :],
                                start=(k == 0),
                                stop=(k == KH - 1),
                            )
                    # relu^2 -> act [128 i-sub, NI, 128 tok] f16.
                    # xs carries a XS_S scale, so pu = XS_S*up; the relu
                    # scale 1/XS_S^2 makes act = relu(up)^2 exactly.
                    r = p_r.tile([P, NI * P], f32, name="r_e")
                    nc.scalar.activation(
                        r[:],
                        pu[:].rearrange("p a b -> p (a b)"),
                        Relu,
                        0.0,
                        1.0 / (XS_S * XS_S),
                        0.0,
                    )
                    act = p_act.tile([P, NI, P], f16, name="act")
                    nc.vector.tensor_tensor(
                        out=act[:].rearrange("p a b -> p (a b)"),
                        in0=pu[:].rearrange("p a b -> p (a b)"),
                        in1=r[:],
                        op=mybir.AluOpType.mult,
                    )
                    if staged is not None:
                        expert_down(staged)
                    staged = (v, act, w2_t)
            if staged is not None:
                expert_down(staged)

            # ---------------- shared down blocks (tail) ------------------
            # depend only on early loads; their stores pack the DMA tail
            # while the final expert's compute drains.
            for mt in range(TB2):
                po = [ps_main.tile([P, 512], f32, name="psA") for _ in range(NH)]
                for j in range(NSQ):
                    for hch in range(NH):
                        nc.tensor.matmul(
                            po[hch][:],
                            ash[:, j, mt * P : (mt + 1) * P],
                            sd_t[:, j, hch * 512 : (hch + 1) * 512],
                            start=(j == 0),
                            stop=(j == NSQ - 1),
                        )
                o_t = p_y.tile([P, H], f16, name="o_t")
                for hch in range(NH):
                    if hch % 2 == 0:
                        nc.vector.tensor_copy(
                            o_t[:, hch * 512 : (hch + 1) * 512], po[hch][:]
                        )
                    else:
                        nc.scalar.activation(
                            o_t[:, hch * 512 : (hch + 1) * 512], po[hch][:], Copy, 0.0
                        )
                nc.gpsimd.dma_start(out[mt * P : (mt + 1) * P, :], o_t[:])

    nc.compile()
    return nc


def _prepare(inputs):
    """Host gate + dispatch: returns (nc, in_maps) ready for SPMD dispatch."""
    import ml_dtypes

    hidden_states = np.asarray(inputs["hidden_states"], dtype=np.float32)
    gate_w = np.asarray(inputs["gate_w"], dtype=np.float32)
    gate_bias = np.asarray(inputs["gate_bias"], dtype=np.float32)
    w1 = np.asarray(inputs["w1"], dtype=np.float32)
    w2 = np.asarray(inputs["w2"], dtype=np.float32)
    shared_up = np.asarray(inputs["shared_up"], dtype=np.float32)
    shared_down = np.asarray(inputs["shared_down"], dtype=np.float32)

    x = hidden_states.reshape(T, H)

    # ---- host gate + dispatch ----
    top_idx, topw = _gate_numpy(x, gate_w, gate_bias)
    sqw = np.sqrt(topw)

    tok_lists = [[] for _ in range(E)]
    scale_lists = [[] for _ in range(E)]
    for kk in range(K):
        for t in range(T):
            e = top_idx[t, kk]
            tok_lists[e].append(t)
            scale_lists[e].append(sqw[t, kk])
    counts = np.array([len(l) for l in tok_lists])
    nslot = max(1, int(np.ceil(counts.max() / P)))

    if nslot not in _PROG_CACHE:
        _PROG_CACHE[nslot] = _build_program(nslot)
    nc = _PROG_CACHE[nslot]

    NV = EL * nslot
    CAP = nslot * P

    in_maps = []
    for c in range(N_CORES):
        xs_b = np.zeros((NV, P, KH, P), ml_dtypes.float8_e3m4)
        for j in range(EL):
            e = c * EL + j
            toks = np.array(tok_lists[e], dtype=np.int64)
            scls = np.array(scale_lists[e], dtype=np.float32)
            n = len(toks)
            assert n <= CAP
            if n:
                xsp = np.zeros((CAP, H), np.float32)
                xsp[:n] = x[toks] * (scls[:, None] * XS_S)
                # [v within expert, p, k, c] = xsp[128v+c, 128k+p]
                xs_b[j * nslot : (j + 1) * nslot] = (
                    xsp.reshape(nslot, P, KH, P)
                    .transpose(0, 3, 2, 1)
                    .astype(ml_dtypes.float8_e3m4)
                )
        w1c = w1[c * EL : (c + 1) * EL]  # [EL, I, H]
        w2c = w2[c * EL : (c + 1) * EL]  # [EL, H, I]
        # w1 blocked: [e, p, k, ci, i] = w1[e, 128ci+i, 128k+p]
        w1_b = w1c.reshape(EL, NI, P, KH, P).transpose(0, 4, 3, 1, 2)
        # w2 blocked: [e, p, ip, h] = w2[e, h, 128ip+p]
        w2_b = w2c.reshape(EL, H, NI, P).transpose(0, 3, 2, 1)
        # shared 2x4 split: token half th, intermediate quarter q
        th, q = c // 4, c % 4
        cs = q * SQ
        # xt blocked: [p, k, t'] = x[512*th + t', 128k+p]
        xt_b = (
            x[th * T2 : (th + 1) * T2]
            .reshape(T2, KH, P)
            .transpose(2, 1, 0)
            .astype(np.float16)
        )
        # su blocked: [p, k, m, i] = shared_up[cs+128m+i, 128k+p]
        su_b = shared_up[cs : cs + SQ].reshape(NSQ, P, KH, P).transpose(3, 2, 0, 1)
        # sd blocked: [p, j, h] = shared_down[h, cs+128j+p]
        sd_b = shared_down[:, cs : cs + SQ].reshape(H, NSQ, P).transpose(2, 1, 0)
        in_maps.append(
            {
                "xt": np.ascontiguousarray(xt_b),
                "xs": np.ascontiguousarray(xs_b).view(np.uint8),
                "w1": np.ascontiguousarray(w1_b.astype(np.float16)),
                "w2": np.ascontiguousarray(w2_b.astype(np.float16)),
                "su": np.ascontiguousarray(su_b.astype(np.float16)),
                "sd": np.ascontiguousarray(sd_b.astype(np.float16)),
            }
        )

    return nc, in_maps, tok_lists, nslot


def _combine(results, tok_lists, nslot, out_shape, out_dtype):
    """Host unshard: sum shared partials + scatter-add routed expert rows."""
    CAP = nslot * P
    acc = np.zeros((T, H), np.float32)
    for c in range(N_CORES):
        th = c // 4
        acc[th * T2 : (th + 1) * T2] += results[c]["out"].astype(np.float32)
    for c in range(N_CORES):
        ya = results[c]["yall"].astype(np.float32)
        for j in range(EL):
            toks = tok_lists[c * EL + j]
            n = len(toks)
            if n:
                acc[toks] += ya[j * CAP : j * CAP + n]
    return acc.reshape(out_shape).astype(out_dtype)


def kernel(**inputs):
    from concourse.bass_utils import run_bass_kernel_spmd

    hidden_states = np.asarray(inputs["hidden_states"], dtype=np.float32)
    nc, in_maps, tok_lists, nslot = _prepare(inputs)
    res = run_bass_kernel_spmd(nc, in_maps, list(range(N_CORES)))
    return _combine(
        res.results, tok_lists, nslot, hidden_states.shape, hidden_states.dtype
    )
